# revision 18
# baseline (speedup 1.0000x reference)
"""MoE-LoRA top-1 dispatch kernel for 8 Trainium2 NeuronCores.

Problem: nn_MoELoRA_19679540150609
  x [4, 2048, 4096] routed per-sample to one of 4 LoRA experts:
  out[b] = (x[b] @ A_e^T) @ B_e^T * 2.0,  e = argmax(router(x[b].mean(S), ...))
  plus a scalar balance loss from the gate probabilities.

Strategy:
  - Router (tiny: 4 samples x 4103x128 MLP) computed on host in f32,
    matching jax semantics (tanh-approximate GELU, softmax, argmax).
  - Heavy work (8.6 GFLOP of matmuls, 256 MiB I/O) sharded over 8 cores:
    core c handles sample b = c//2, sequence half c%2 (1024 rows each).
  - Each core receives x-shard pre-transposed [4096, 1024] (host transpose;
    the PE contracts over the partition dim so x^T is required on-chip and
    fp32 DMA-transpose doesn't exist), A_e^T * 2.0 [4096, 64] (LoRA scaling
    folded in, exact since 2.0 is a power of two) and B_e^T [64, 4096].
  - On-chip per core: xr^T = (2A_e^T)^T @ x^T  accumulated over 32 K-chunks
    of 128, then out = xr @ B_e^T per 128-row subtile, streamed back.
"""

import numpy as np

import concourse.bacc as bacc
import concourse.mybir as mybir
import concourse.tile as tile
from concourse.bass_utils import run_bass_kernel_spmd

B, S, D = 4, 2048, 4096
E, R, OUT = 4, 64, 4096
HID = 128
SCALING = 2.0
BALANCE_COEFF = 0.01

N_CORES = 8
S_SHARD = S * B // N_CORES  # 1024 rows of x per core
P = 128
KC = D // P          # 32 contraction chunks for x @ A^T
ST = 512             # S-tile (matmul free dim)
NT = S_SHARD // ST   # 2 S-tiles per core
MSUB = ST // P       # 4 psum-row subtiles per S-tile
OC = OUT // 512      # 8 output column chunks

# Matmul operand dtype: float32 is exact; float32r is ~1.5e-4 and 4x faster.
MM_DT = mybir.dt.float32


def _build_kernel(n_cores: int):
    """Both matmuls have a 64-wide dim (R) that would leave half the 128x128
    PE array idle. The two 512-row S-tiles of the core's shard are packed
    into the two array halves via tile_position: mm1 puts S-tile 0 in
    columns 0-63 / S-tile 1 in columns 64-127 (outputs land on psum
    partitions 0-63 / 64-127 of the same bank), mm2 puts them in rows
    0-63 / 64-127 (B^T duplicated across both partition halves). Everything
    stays partition-aligned, fp32-exact, and PE time halves twice."""
    nc = bacc.Bacc(
        "TRN2", target_bir_lowering=False, debug=False, num_devices=n_cores
    )
    xt = nc.dram_tensor("xt", [D, S_SHARD], mybir.dt.float32, kind="ExternalInput")
    at = nc.dram_tensor("at", [D, R], mybir.dt.float32, kind="ExternalInput")
    bt = nc.dram_tensor("bt", [R, OUT], mybir.dt.float32, kind="ExternalInput")
    out = nc.dram_tensor(
        "out", [S_SHARD, OUT], mybir.dt.float32, kind="ExternalOutput"
    )

    KG = 8             # xt k-groups per pair (1 MiB DMAs, 2KB runs)
    KPG = KC // KG     # k-chunks per group
    NPAIR = 2          # compute pipelined over s-pairs
    SP = S_SHARD // NPAIR   # 512 rows per pair (2 packed tiles of 256)
    STP = SP // 2      # 256: matmul free dim for mm1

    with tile.TileContext(nc) as tc:
        with (
            tc.tile_pool(name="const", bufs=1) as const,
            tc.tile_pool(name="xload", bufs=1) as xload,
            tc.tile_pool(name="xr", bufs=2) as xrp,
            # deep staging so out-DMA keeps draining through mm1 phases
            tc.tile_pool(name="ostage", bufs=6) as ostage,
            tc.tile_pool(name="ps1", bufs=2, space="PSUM") as ps1,
            tc.tile_pool(name="ps2", bufs=3, space="PSUM") as ps2,
        ):
            # All DMAs ride the SP HWDGE ring, which drains FIFO: inputs
            # are issued first (pair 0 before pair 1) and outputs queue
            # behind them, so HBM runs saturated start-to-finish with no
            # in/out bandwidth competition. The small A^T load goes via
            # SWDGE so it doesn't delay the first x group.
            at_sb = const.tile([P, KC, R], MM_DT)
            bt_sb = const.tile([P, OUT], MM_DT)
            nc.gpsimd.dma_start(at_sb[:], at.rearrange("(c p) r -> p c r", p=P))

            xt_t = {}
            for h in range(NPAIR):
                for g in range(KG):
                    xg = xload.tile([P, KPG, SP], MM_DT, tag=f"xt{h}_{g}")
                    nc.sync.dma_start(
                        xg[:],
                        xt[
                            g * KPG * P : (g + 1) * KPG * P,
                            h * SP : (h + 1) * SP,
                        ].rearrange("(c p) s -> p c s", p=P),
                    )
                    xt_t[h, g] = xg
                if h == 0:
                    nc.sync.dma_start(bt_sb[0:R, :], bt[:])
                    # duplicate across the second partition half (SBUF fabric)
                    nc.sync.dma_start(bt_sb[R : 2 * R, :], bt_sb[0:R, :])

            def mm1_group(h, g, acc):
                """One k-group of mm1 for pair h: 16 column-packed matmuls."""
                for kk in range(KPG):
                    k = g * KPG + kk
                    xk = xt_t[h, g][:, kk, :]
                    nc.tensor.matmul(
                        acc[0:R, :],
                        at_sb[:, k, :],
                        xk[:, 0:STP],
                        start=(k == 0),
                        stop=(k == KC - 1),
                        tile_position=(0, 0),
                    )
                    nc.tensor.matmul(
                        acc[R : 2 * R, :],
                        at_sb[:, k, :],
                        xk[:, STP : 2 * STP],
                        start=(k == 0),
                        stop=(k == KC - 1),
                        tile_position=(0, R),
                    )

            def mm2_block(h, m, n2, xr_sb):
                """One (m, n2) block of mm2 for pair h: 4 row-packed matmuls,
                staged 1024 wide per out-DMA for 4KB DRAM runs."""
                stg0 = ostage.tile([P, 2, 512], mybir.dt.float32, tag="st0")
                stg1 = ostage.tile([P, 2, 512], mybir.dt.float32, tag="st1")
                stg = [stg0, stg1]
                for j in range(2):
                    n = 2 * n2 + j
                    po0 = ps2.tile([P, 512], mybir.dt.float32, tag="po0")
                    po1 = ps2.tile([P, 512], mybir.dt.float32, tag="po1")
                    nc.tensor.matmul(
                        po0[:],
                        xr_sb[0:R, m * P : (m + 1) * P],
                        bt_sb[0:R, n * 512 : (n + 1) * 512],
                        start=True,
                        stop=True,
                        tile_position=(0, 0),
                    )
                    nc.tensor.matmul(
                        po1[:],
                        xr_sb[R : 2 * R, m * P : (m + 1) * P],
                        bt_sb[R : 2 * R, n * 512 : (n + 1) * 512],
                        start=True,
                        stop=True,
                        tile_position=(R, 0),
                    )
                    nc.vector.tensor_copy(stg[0][:, j, :], po0[:])
                    nc.vector.tensor_copy(stg[1][:, j, :], po1[:])
                for t in range(2):
                    row0 = h * SP + t * STP + m * P
                    nc.sync.dma_start(
                        out[row0 : row0 + P, n2 * 1024 : (n2 + 1) * 1024],
                        stg[t].rearrange("p a b -> p (a b)"),
                    )

            def xr_copy(acc):
                xr_sb = xrp.tile([P, STP], MM_DT)
                nc.vector.tensor_copy(xr_sb[:], acc[:])
                return xr_sb

            # Pair 0: mm1 paced by its input DMAs.
            acc0 = ps1.tile([P, STP], mybir.dt.float32, tag="acc")
            for g in range(KG):
                mm1_group(0, g, acc0)
            xr0 = xr_copy(acc0)

            # Pair 0's mm2 with pair 1's mm1 k-groups woven in, so after the
            # last input byte lands only mm2(1) remains on the PE.
            acc1 = ps1.tile([P, STP], mybir.dt.float32, tag="acc")
            blocks = [(m, n2) for m in range(STP // P) for n2 in range(OC // 2)]
            for i, (m, n2) in enumerate(blocks):
                mm2_block(0, m, n2, xr0)
                if i < KG:
                    mm1_group(1, i, acc1)
            xr1 = xr_copy(acc1)

            for m, n2 in blocks:
                mm2_block(1, m, n2, xr1)
    nc.compile()
    return nc


_NC_CACHE: dict[int, object] = {}


def _get_nc(n_cores: int):
    if n_cores not in _NC_CACHE:
        _NC_CACHE[n_cores] = _build_kernel(n_cores)
    return _NC_CACHE[n_cores]


def _route_host(x, reliability_vec, regime_vec, w1, b1, w2, b2):
    """Router forward in f32 numpy, matching the jax reference."""
    x32 = np.asarray(x, np.float32)
    x_pooled = x32.mean(axis=1)  # [B, D]
    router_in = np.concatenate(
        [x_pooled, np.asarray(reliability_vec, np.float32),
         np.asarray(regime_vec, np.float32)], axis=-1
    ).astype(np.float32)
    h = router_in @ np.asarray(w1, np.float32) + np.asarray(b1, np.float32)
    # jax.nn.gelu default is the tanh approximation
    h32 = h.astype(np.float32)
    g = 0.5 * h32 * (
        1.0 + np.tanh(
            np.float32(np.sqrt(2.0 / np.pi))
            * (h32 + np.float32(0.044715) * h32 * h32 * h32)
        )
    )
    logits = g.astype(np.float32) @ np.asarray(w2, np.float32) + np.asarray(
        b2, np.float32
    )
    logits = logits.astype(np.float32)
    zmax = logits.max(axis=-1, keepdims=True)
    ez = np.exp(logits - zmax)
    probs = ez / ez.sum(axis=-1, keepdims=True)
    top = np.argmax(probs, axis=-1)
    avg_gate = probs.mean(axis=0)
    balance = np.float32(
        BALANCE_COEFF * (E * np.sum(avg_gate.astype(np.float32) ** 2))
    )
    return top, balance


def kernel(x, reliability_vec, regime_vec, lora_A, lora_B, w1, b1, w2, b2):
    x = np.asarray(x, np.float32)
    lora_A = np.asarray(lora_A, np.float32)
    lora_B = np.asarray(lora_B, np.float32)

    top, balance = _route_host(
        x, reliability_vec, regime_vec, w1, b1, w2, b2
    )

    # Per-expert host prep (tiny tensors): A_e^T with LoRA scaling folded in
    # (exact, power of two) and B_e^T.
    at_by_e = {}
    bt_by_e = {}
    for e in set(int(t) for t in top):
        at_by_e[e] = np.ascontiguousarray(lora_A[e].T) * np.float32(SCALING)
        bt_by_e[e] = np.ascontiguousarray(lora_B[e].T)

    in_maps = []
    for c in range(N_CORES):
        b, half = divmod(c, N_CORES // B)
        e = int(top[b])
        xt_c = np.ascontiguousarray(
            x[b, half * S_SHARD : (half + 1) * S_SHARD, :].T
        )
        in_maps.append({"xt": xt_c, "at": at_by_e[e], "bt": bt_by_e[e]})

    nc = _get_nc(N_CORES)
    res = run_bass_kernel_spmd(nc, in_maps, core_ids=list(range(N_CORES)))

    out = np.empty((B, S, OUT), np.float32)
    for c in range(N_CORES):
        b, half = divmod(c, N_CORES // B)
        out[b, half * S_SHARD : (half + 1) * S_SHARD, :] = res.results[c]["out"]

    return out, balance


# revision 19
# speedup vs baseline: 1.0978x; 1.0978x over previous
"""MoE-LoRA top-1 dispatch kernel for 8 Trainium2 NeuronCores.

Problem: nn_MoELoRA_19679540150609
  x [4, 2048, 4096] routed per-sample to one of 4 LoRA experts:
  out[b] = (x[b] @ A_e^T) @ B_e^T * 2.0,  e = argmax(router(x[b].mean(S), ...))
  plus a scalar balance loss from the gate probabilities.

Strategy:
  - Router (tiny: 4 samples x 4103x128 MLP) computed on host in f32,
    matching jax semantics (tanh-approximate GELU, softmax, argmax).
  - Heavy work (8.6 GFLOP of matmuls, 256 MiB I/O) sharded over 8 cores:
    core c handles sample b = c//2, sequence half c%2 (1024 rows each).
  - Each core receives x-shard pre-transposed [4096, 1024] (host transpose;
    the PE contracts over the partition dim so x^T is required on-chip and
    fp32 DMA-transpose doesn't exist), A_e^T * 2.0 [4096, 64] (LoRA scaling
    folded in, exact since 2.0 is a power of two) and B_e^T [64, 4096].
  - On-chip per core: xr^T = (2A_e^T)^T @ x^T  accumulated over 32 K-chunks
    of 128, then out = xr @ B_e^T per 128-row subtile, streamed back.
"""

import numpy as np

import concourse.bacc as bacc
import concourse.mybir as mybir
import concourse.tile as tile
from concourse.bass_utils import run_bass_kernel_spmd

B, S, D = 4, 2048, 4096
E, R, OUT = 4, 64, 4096
HID = 128
SCALING = 2.0
BALANCE_COEFF = 0.01

N_CORES = 8
S_SHARD = S * B // N_CORES  # 1024 rows of x per core
P = 128
KC = D // P          # 32 contraction chunks for x @ A^T
ST = 512             # S-tile (matmul free dim)
NT = S_SHARD // ST   # 2 S-tiles per core
MSUB = ST // P       # 4 psum-row subtiles per S-tile
OC = OUT // 512      # 8 output column chunks

# Matmul operand dtype: float32 is exact; float32r is ~1.5e-4 and 4x faster.
MM_DT = mybir.dt.float32


def _build_kernel(n_cores: int):
    """Both matmuls have a 64-wide dim (R) that would leave half the 128x128
    PE array idle. The two 512-row S-tiles of the core's shard are packed
    into the two array halves via tile_position: mm1 puts S-tile 0 in
    columns 0-63 / S-tile 1 in columns 64-127 (outputs land on psum
    partitions 0-63 / 64-127 of the same bank), mm2 puts them in rows
    0-63 / 64-127 (B^T duplicated across both partition halves). Everything
    stays partition-aligned, fp32-exact, and PE time halves twice."""
    nc = bacc.Bacc(
        "TRN2", target_bir_lowering=False, debug=False, num_devices=n_cores
    )
    xt = nc.dram_tensor("xt", [D, S_SHARD], mybir.dt.float32, kind="ExternalInput")
    at = nc.dram_tensor("at", [D, R], mybir.dt.float32, kind="ExternalInput")
    bt = nc.dram_tensor("bt", [R, OUT], mybir.dt.float32, kind="ExternalInput")
    out = nc.dram_tensor(
        "out", [S_SHARD, OUT], mybir.dt.float32, kind="ExternalOutput"
    )

    KG = 8             # xt k-groups per pair (1 MiB DMAs, 2KB runs)
    KPG = KC // KG     # k-chunks per group
    NPAIR = 2          # compute pipelined over s-pairs
    SP = S_SHARD // NPAIR   # 512 rows per pair (2 packed tiles of 256)
    STP = SP // 2      # 256: matmul free dim for mm1

    with tile.TileContext(nc) as tc:
        with (
            tc.tile_pool(name="const", bufs=1) as const,
            tc.tile_pool(name="xload", bufs=1) as xload,
            tc.tile_pool(name="xr", bufs=2) as xrp,
            # deep staging so out-DMA keeps draining through mm1 phases
            tc.tile_pool(name="ostage", bufs=6) as ostage,
            tc.tile_pool(name="ps1", bufs=2, space="PSUM") as ps1,
            tc.tile_pool(name="ps2", bufs=3, space="PSUM") as ps2,
        ):
            # All DMAs ride the SP HWDGE ring, which drains FIFO: inputs
            # are issued first (pair 0 before pair 1) and outputs queue
            # behind them, so HBM runs saturated start-to-finish with no
            # in/out bandwidth competition. The small A^T load goes via
            # SWDGE so it doesn't delay the first x group.
            at_sb = const.tile([P, KC, R], MM_DT)
            bt_sb = const.tile([P, OUT], MM_DT)
            nc.gpsimd.dma_start(at_sb[:], at.rearrange("(c p) r -> p c r", p=P))

            xt_t = {}
            for h in range(NPAIR):
                for g in range(KG):
                    xg = xload.tile([P, KPG, SP], MM_DT, tag=f"xt{h}_{g}")
                    nc.sync.dma_start(
                        xg[:],
                        xt[
                            g * KPG * P : (g + 1) * KPG * P,
                            h * SP : (h + 1) * SP,
                        ].rearrange("(c p) s -> p c s", p=P),
                    )
                    xt_t[h, g] = xg
                if h == 0:
                    nc.sync.dma_start(bt_sb[0:R, :], bt[:])
                    # duplicate across the second partition half (SBUF fabric,
                    # via SWDGE so it takes no SP-ring slot)
                    nc.gpsimd.dma_start(bt_sb[R : 2 * R, :], bt_sb[0:R, :])

            def mm1_group(h, g, acc):
                """One k-group of mm1 for pair h: 16 column-packed matmuls."""
                for kk in range(KPG):
                    k = g * KPG + kk
                    xk = xt_t[h, g][:, kk, :]
                    nc.tensor.matmul(
                        acc[0:R, :],
                        at_sb[:, k, :],
                        xk[:, 0:STP],
                        start=(k == 0),
                        stop=(k == KC - 1),
                        tile_position=(0, 0),
                    )
                    nc.tensor.matmul(
                        acc[R : 2 * R, :],
                        at_sb[:, k, :],
                        xk[:, STP : 2 * STP],
                        start=(k == 0),
                        stop=(k == KC - 1),
                        tile_position=(0, R),
                    )

            def mm2_block(h, m, n2, xr_sb):
                """One (m, n2) block of mm2 for pair h: 4 row-packed matmuls,
                staged 1024 wide per out-DMA for 4KB DRAM runs."""
                stg0 = ostage.tile([P, 2, 512], mybir.dt.float32, tag="st0")
                stg1 = ostage.tile([P, 2, 512], mybir.dt.float32, tag="st1")
                stg = [stg0, stg1]
                for j in range(2):
                    n = 2 * n2 + j
                    po0 = ps2.tile([P, 512], mybir.dt.float32, tag="po0")
                    po1 = ps2.tile([P, 512], mybir.dt.float32, tag="po1")
                    nc.tensor.matmul(
                        po0[:],
                        xr_sb[0:R, m * P : (m + 1) * P],
                        bt_sb[0:R, n * 512 : (n + 1) * 512],
                        start=True,
                        stop=True,
                        tile_position=(0, 0),
                    )
                    nc.tensor.matmul(
                        po1[:],
                        xr_sb[R : 2 * R, m * P : (m + 1) * P],
                        bt_sb[R : 2 * R, n * 512 : (n + 1) * 512],
                        start=True,
                        stop=True,
                        tile_position=(R, 0),
                    )
                    nc.vector.tensor_copy(stg[0][:, j, :], po0[:])
                    nc.vector.tensor_copy(stg[1][:, j, :], po1[:])
                for t in range(2):
                    row0 = h * SP + t * STP + m * P
                    nc.sync.dma_start(
                        out[row0 : row0 + P, n2 * 1024 : (n2 + 1) * 1024],
                        stg[t].rearrange("p a b -> p (a b)"),
                    )

            def xr_copy(acc):
                xr_sb = xrp.tile([P, STP], MM_DT)
                nc.vector.tensor_copy(xr_sb[:], acc[:])
                return xr_sb

            # Pair 0: mm1 paced by its input DMAs.
            acc0 = ps1.tile([P, STP], mybir.dt.float32, tag="acc")
            for g in range(KG):
                mm1_group(0, g, acc0)
            xr0 = xr_copy(acc0)

            # Pair 0's mm2 with pair 1's mm1 k-groups woven in, so after the
            # last input byte lands only mm2(1) remains on the PE.
            acc1 = ps1.tile([P, STP], mybir.dt.float32, tag="acc")
            blocks = [(m, n2) for m in range(STP // P) for n2 in range(OC // 2)]
            for i, (m, n2) in enumerate(blocks):
                mm2_block(0, m, n2, xr0)
                if i < KG:
                    mm1_group(1, i, acc1)
            xr1 = xr_copy(acc1)

            for m, n2 in blocks:
                mm2_block(1, m, n2, xr1)
    nc.compile()
    return nc


_NC_CACHE: dict[int, object] = {}


def _get_nc(n_cores: int):
    if n_cores not in _NC_CACHE:
        _NC_CACHE[n_cores] = _build_kernel(n_cores)
    return _NC_CACHE[n_cores]


def _route_host(x, reliability_vec, regime_vec, w1, b1, w2, b2):
    """Router forward in f32 numpy, matching the jax reference."""
    x32 = np.asarray(x, np.float32)
    x_pooled = x32.mean(axis=1)  # [B, D]
    router_in = np.concatenate(
        [x_pooled, np.asarray(reliability_vec, np.float32),
         np.asarray(regime_vec, np.float32)], axis=-1
    ).astype(np.float32)
    h = router_in @ np.asarray(w1, np.float32) + np.asarray(b1, np.float32)
    # jax.nn.gelu default is the tanh approximation
    h32 = h.astype(np.float32)
    g = 0.5 * h32 * (
        1.0 + np.tanh(
            np.float32(np.sqrt(2.0 / np.pi))
            * (h32 + np.float32(0.044715) * h32 * h32 * h32)
        )
    )
    logits = g.astype(np.float32) @ np.asarray(w2, np.float32) + np.asarray(
        b2, np.float32
    )
    logits = logits.astype(np.float32)
    zmax = logits.max(axis=-1, keepdims=True)
    ez = np.exp(logits - zmax)
    probs = ez / ez.sum(axis=-1, keepdims=True)
    top = np.argmax(probs, axis=-1)
    avg_gate = probs.mean(axis=0)
    balance = np.float32(
        BALANCE_COEFF * (E * np.sum(avg_gate.astype(np.float32) ** 2))
    )
    return top, balance


def kernel(x, reliability_vec, regime_vec, lora_A, lora_B, w1, b1, w2, b2):
    x = np.asarray(x, np.float32)
    lora_A = np.asarray(lora_A, np.float32)
    lora_B = np.asarray(lora_B, np.float32)

    top, balance = _route_host(
        x, reliability_vec, regime_vec, w1, b1, w2, b2
    )

    # Per-expert host prep (tiny tensors): A_e^T with LoRA scaling folded in
    # (exact, power of two) and B_e^T.
    at_by_e = {}
    bt_by_e = {}
    for e in set(int(t) for t in top):
        at_by_e[e] = np.ascontiguousarray(lora_A[e].T) * np.float32(SCALING)
        bt_by_e[e] = np.ascontiguousarray(lora_B[e].T)

    in_maps = []
    for c in range(N_CORES):
        b, half = divmod(c, N_CORES // B)
        e = int(top[b])
        xt_c = np.ascontiguousarray(
            x[b, half * S_SHARD : (half + 1) * S_SHARD, :].T
        )
        in_maps.append({"xt": xt_c, "at": at_by_e[e], "bt": bt_by_e[e]})

    nc = _get_nc(N_CORES)
    res = run_bass_kernel_spmd(nc, in_maps, core_ids=list(range(N_CORES)))

    out = np.empty((B, S, OUT), np.float32)
    for c in range(N_CORES):
        b, half = divmod(c, N_CORES // B)
        out[b, half * S_SHARD : (half + 1) * S_SHARD, :] = res.results[c]["out"]

    return out, balance


# revision 21
# speedup vs baseline: 1.1034x; 1.0051x over previous
"""MoE-LoRA top-1 dispatch kernel for 8 Trainium2 NeuronCores.

Problem: nn_MoELoRA_19679540150609
  x [4, 2048, 4096] routed per-sample to one of 4 LoRA experts:
  out[b] = (x[b] @ A_e^T) @ B_e^T * 2.0,  e = argmax(router(x[b].mean(S), ...))
  plus a scalar balance loss from the gate probabilities.

Strategy:
  - Router (tiny: 4 samples x 4103x128 MLP) computed on host in f32,
    matching jax semantics (tanh-approximate GELU, softmax, argmax).
  - Heavy work (8.6 GFLOP of matmuls, 256 MiB I/O) sharded over 8 cores:
    core c handles sample b = c//2, sequence half c%2 (1024 rows each).
  - Each core receives x-shard pre-transposed [4096, 1024] (host transpose;
    the PE contracts over the partition dim so x^T is required on-chip and
    fp32 DMA-transpose doesn't exist), A_e^T * 2.0 [4096, 64] (LoRA scaling
    folded in, exact since 2.0 is a power of two) and B_e^T [64, 4096].
  - On-chip per core: xr^T = (2A_e^T)^T @ x^T  accumulated over 32 K-chunks
    of 128, then out = xr @ B_e^T per 128-row subtile, streamed back.
"""

import numpy as np

import concourse.bacc as bacc
import concourse.mybir as mybir
import concourse.tile as tile
from concourse.bass_utils import run_bass_kernel_spmd

B, S, D = 4, 2048, 4096
E, R, OUT = 4, 64, 4096
HID = 128
SCALING = 2.0
BALANCE_COEFF = 0.01

N_CORES = 8
S_SHARD = S * B // N_CORES  # 1024 rows of x per core
P = 128
KC = D // P          # 32 contraction chunks for x @ A^T
ST = 512             # S-tile (matmul free dim)
NT = S_SHARD // ST   # 2 S-tiles per core
MSUB = ST // P       # 4 psum-row subtiles per S-tile
OC = OUT // 512      # 8 output column chunks

# Matmul operand dtype: float32 is exact; float32r is ~1.5e-4 and 4x faster.
MM_DT = mybir.dt.float32

N_PAIR = 2           # pipelined s-pairs per core
K_GRP = 8            # k-groups per pair (one input DMA each)


def _build_kernel(n_cores: int):
    """Both matmuls have a 64-wide dim (R) that would leave half the 128x128
    PE array idle. The two 512-row S-tiles of the core's shard are packed
    into the two array halves via tile_position: mm1 puts S-tile 0 in
    columns 0-63 / S-tile 1 in columns 64-127 (outputs land on psum
    partitions 0-63 / 64-127 of the same bank), mm2 puts them in rows
    0-63 / 64-127 (B^T duplicated across both partition halves). Everything
    stays partition-aligned, fp32-exact, and PE time halves twice."""
    nc = bacc.Bacc(
        "TRN2", target_bir_lowering=False, debug=False, num_devices=n_cores
    )
    # xt and at arrive host-packed in SBUF-native layout, so every input
    # DMA reads 8-16KB contiguous per partition (max descriptor efficiency).
    xt = nc.dram_tensor(
        "xt", [N_PAIR, K_GRP, P, KC // K_GRP, S_SHARD // N_PAIR],
        mybir.dt.float32, kind="ExternalInput",
    )
    at = nc.dram_tensor("at", [P, KC, R], mybir.dt.float32, kind="ExternalInput")
    bt = nc.dram_tensor("bt", [R, OUT], mybir.dt.float32, kind="ExternalInput")
    out = nc.dram_tensor(
        "out", [S_SHARD, OUT], mybir.dt.float32, kind="ExternalOutput"
    )

    KG = K_GRP         # xt k-groups per pair (1 MiB DMAs, 16KB/partition runs)
    KPG = KC // KG     # k-chunks per group
    NPAIR = N_PAIR     # compute pipelined over s-pairs
    SP = S_SHARD // NPAIR   # 512 rows per pair (2 packed tiles of 256)
    STP = SP // 2      # 256: matmul free dim for mm1

    with tile.TileContext(nc) as tc:
        with (
            tc.tile_pool(name="const", bufs=1) as const,
            tc.tile_pool(name="xload", bufs=1) as xload,
            tc.tile_pool(name="xr", bufs=2) as xrp,
            # deep staging so out-DMA keeps draining through mm1 phases
            tc.tile_pool(name="ostage", bufs=6) as ostage,
            tc.tile_pool(name="ps1", bufs=2, space="PSUM") as ps1,
            tc.tile_pool(name="ps2", bufs=3, space="PSUM") as ps2,
        ):
            # All DMAs ride the SP HWDGE ring, which drains FIFO: inputs
            # are issued first (pair 0 before pair 1) and outputs queue
            # behind them, so HBM runs saturated start-to-finish with no
            # in/out bandwidth competition. The small A^T load goes via
            # SWDGE so it doesn't delay the first x group.
            at_sb = const.tile([P, KC, R], MM_DT)
            bt_sb = const.tile([P, OUT], MM_DT)
            nc.gpsimd.dma_start(at_sb[:], at[:])

            xt_t = {}
            for h in range(NPAIR):
                for g in range(KG):
                    xg = xload.tile([P, KPG, SP], MM_DT, tag=f"xt{h}_{g}")
                    nc.sync.dma_start(xg[:], xt[h, g])
                    xt_t[h, g] = xg
                if h == 0:
                    nc.sync.dma_start(bt_sb[0:R, :], bt[:])
                    # duplicate across the second partition half (SBUF fabric,
                    # via SWDGE so it takes no SP-ring slot)
                    nc.gpsimd.dma_start(bt_sb[R : 2 * R, :], bt_sb[0:R, :])

            def mm1_group(h, g, acc):
                """One k-group of mm1 for pair h: 16 column-packed matmuls."""
                for kk in range(KPG):
                    k = g * KPG + kk
                    xk = xt_t[h, g][:, kk, :]
                    nc.tensor.matmul(
                        acc[0:R, :],
                        at_sb[:, k, :],
                        xk[:, 0:STP],
                        start=(k == 0),
                        stop=(k == KC - 1),
                        tile_position=(0, 0),
                    )
                    nc.tensor.matmul(
                        acc[R : 2 * R, :],
                        at_sb[:, k, :],
                        xk[:, STP : 2 * STP],
                        start=(k == 0),
                        stop=(k == KC - 1),
                        tile_position=(0, R),
                    )

            def mm2_block(h, m, n2, xr_sb):
                """One (m, n2) block of mm2 for pair h: 4 row-packed matmuls,
                staged 1024 wide per out-DMA for 4KB DRAM runs."""
                stg0 = ostage.tile([P, 2, 512], mybir.dt.float32, tag="st0")
                stg1 = ostage.tile([P, 2, 512], mybir.dt.float32, tag="st1")
                stg = [stg0, stg1]
                for j in range(2):
                    n = 2 * n2 + j
                    po0 = ps2.tile([P, 512], mybir.dt.float32, tag="po0")
                    po1 = ps2.tile([P, 512], mybir.dt.float32, tag="po1")
                    nc.tensor.matmul(
                        po0[:],
                        xr_sb[0:R, m * P : (m + 1) * P],
                        bt_sb[0:R, n * 512 : (n + 1) * 512],
                        start=True,
                        stop=True,
                        tile_position=(0, 0),
                    )
                    nc.tensor.matmul(
                        po1[:],
                        xr_sb[R : 2 * R, m * P : (m + 1) * P],
                        bt_sb[R : 2 * R, n * 512 : (n + 1) * 512],
                        start=True,
                        stop=True,
                        tile_position=(R, 0),
                    )
                    nc.vector.tensor_copy(stg[0][:, j, :], po0[:])
                    nc.vector.tensor_copy(stg[1][:, j, :], po1[:])
                for t in range(2):
                    row0 = h * SP + t * STP + m * P
                    nc.sync.dma_start(
                        out[row0 : row0 + P, n2 * 1024 : (n2 + 1) * 1024],
                        stg[t].rearrange("p a b -> p (a b)"),
                    )

            def xr_copy(acc):
                xr_sb = xrp.tile([P, STP], MM_DT)
                nc.vector.tensor_copy(xr_sb[:], acc[:])
                return xr_sb

            # Pair 0: mm1 paced by its input DMAs.
            acc0 = ps1.tile([P, STP], mybir.dt.float32, tag="acc")
            for g in range(KG):
                mm1_group(0, g, acc0)
            xr0 = xr_copy(acc0)

            # Pair 0's mm2 with pair 1's mm1 k-groups woven in, so after the
            # last input byte lands only mm2(1) remains on the PE.
            acc1 = ps1.tile([P, STP], mybir.dt.float32, tag="acc")
            blocks = [(m, n2) for m in range(STP // P) for n2 in range(OC // 2)]
            for i, (m, n2) in enumerate(blocks):
                mm2_block(0, m, n2, xr0)
                if i < KG:
                    mm1_group(1, i, acc1)
            xr1 = xr_copy(acc1)

            for m, n2 in blocks:
                mm2_block(1, m, n2, xr1)
    nc.compile()
    return nc


_NC_CACHE: dict[int, object] = {}


def _get_nc(n_cores: int):
    if n_cores not in _NC_CACHE:
        _NC_CACHE[n_cores] = _build_kernel(n_cores)
    return _NC_CACHE[n_cores]


def _route_host(x, reliability_vec, regime_vec, w1, b1, w2, b2):
    """Router forward in f32 numpy, matching the jax reference."""
    x32 = np.asarray(x, np.float32)
    x_pooled = x32.mean(axis=1)  # [B, D]
    router_in = np.concatenate(
        [x_pooled, np.asarray(reliability_vec, np.float32),
         np.asarray(regime_vec, np.float32)], axis=-1
    ).astype(np.float32)
    h = router_in @ np.asarray(w1, np.float32) + np.asarray(b1, np.float32)
    # jax.nn.gelu default is the tanh approximation
    h32 = h.astype(np.float32)
    g = 0.5 * h32 * (
        1.0 + np.tanh(
            np.float32(np.sqrt(2.0 / np.pi))
            * (h32 + np.float32(0.044715) * h32 * h32 * h32)
        )
    )
    logits = g.astype(np.float32) @ np.asarray(w2, np.float32) + np.asarray(
        b2, np.float32
    )
    logits = logits.astype(np.float32)
    zmax = logits.max(axis=-1, keepdims=True)
    ez = np.exp(logits - zmax)
    probs = ez / ez.sum(axis=-1, keepdims=True)
    top = np.argmax(probs, axis=-1)
    avg_gate = probs.mean(axis=0)
    balance = np.float32(
        BALANCE_COEFF * (E * np.sum(avg_gate.astype(np.float32) ** 2))
    )
    return top, balance


def _pack_xt(x_shard):
    """x-shard [S_SHARD, D] -> SBUF-native [NPAIR, KG, 128, KPG, SP]:
    element [h, g, p, c, s] = x_shard[h*SP + s, (g*KPG + c)*128 + p]."""
    SP = S_SHARD // N_PAIR
    KPG = KC // K_GRP
    v = x_shard.reshape(N_PAIR, SP, K_GRP, KPG, P)
    return np.ascontiguousarray(v.transpose(0, 2, 4, 3, 1))


def _pack_at(a):
    """A_e [R, D] (scaled) -> [128, KC, R]: [p, c, r] = A^T[c*128+p, r]."""
    v = a.T.reshape(KC, P, R)
    return np.ascontiguousarray(v.transpose(1, 0, 2))


def kernel(x, reliability_vec, regime_vec, lora_A, lora_B, w1, b1, w2, b2):
    x = np.asarray(x, np.float32)
    lora_A = np.asarray(lora_A, np.float32)
    lora_B = np.asarray(lora_B, np.float32)

    top, balance = _route_host(
        x, reliability_vec, regime_vec, w1, b1, w2, b2
    )

    # Per-expert host prep (tiny tensors): A_e^T with LoRA scaling folded in
    # (exact, power of two) and B_e^T.
    at_by_e = {}
    bt_by_e = {}
    for e in set(int(t) for t in top):
        at_by_e[e] = _pack_at(lora_A[e] * np.float32(SCALING))
        bt_by_e[e] = np.ascontiguousarray(lora_B[e].T)

    in_maps = []
    for c in range(N_CORES):
        b, half = divmod(c, N_CORES // B)
        e = int(top[b])
        xt_c = _pack_xt(x[b, half * S_SHARD : (half + 1) * S_SHARD, :])
        in_maps.append({"xt": xt_c, "at": at_by_e[e], "bt": bt_by_e[e]})

    nc = _get_nc(N_CORES)
    res = run_bass_kernel_spmd(nc, in_maps, core_ids=list(range(N_CORES)))

    out = np.empty((B, S, OUT), np.float32)
    for c in range(N_CORES):
        b, half = divmod(c, N_CORES // B)
        out[b, half * S_SHARD : (half + 1) * S_SHARD, :] = res.results[c]["out"]

    return out, balance


# revision 22
# speedup vs baseline: 1.1175x; 1.0128x over previous
"""MoE-LoRA top-1 dispatch kernel for 8 Trainium2 NeuronCores.

Problem: nn_MoELoRA_19679540150609
  x [4, 2048, 4096] routed per-sample to one of 4 LoRA experts:
  out[b] = (x[b] @ A_e^T) @ B_e^T * 2.0,  e = argmax(router(x[b].mean(S), ...))
  plus a scalar balance loss from the gate probabilities.

Strategy:
  - Router (tiny: 4 samples x 4103x128 MLP) computed on host in f32,
    matching jax semantics (tanh-approximate GELU, softmax, argmax).
  - Heavy work (8.6 GFLOP of matmuls, 256 MiB I/O) sharded over 8 cores:
    core c handles sample b = c//2, sequence half c%2 (1024 rows each).
  - Each core receives x-shard pre-transposed [4096, 1024] (host transpose;
    the PE contracts over the partition dim so x^T is required on-chip and
    fp32 DMA-transpose doesn't exist), A_e^T * 2.0 [4096, 64] (LoRA scaling
    folded in, exact since 2.0 is a power of two) and B_e^T [64, 4096].
  - On-chip per core: xr^T = (2A_e^T)^T @ x^T  accumulated over 32 K-chunks
    of 128, then out = xr @ B_e^T per 128-row subtile, streamed back.
"""

import numpy as np

import concourse.bacc as bacc
import concourse.mybir as mybir
import concourse.tile as tile
from concourse.bass_utils import run_bass_kernel_spmd

B, S, D = 4, 2048, 4096
E, R, OUT = 4, 64, 4096
HID = 128
SCALING = 2.0
BALANCE_COEFF = 0.01

N_CORES = 8
S_SHARD = S * B // N_CORES  # 1024 rows of x per core
P = 128
KC = D // P          # 32 contraction chunks for x @ A^T
ST = 512             # S-tile (matmul free dim)
NT = S_SHARD // ST   # 2 S-tiles per core
MSUB = ST // P       # 4 psum-row subtiles per S-tile
OC = OUT // 512      # 8 output column chunks

# Matmul operand dtype: float32 is exact; float32r is ~1.5e-4 and 4x faster.
MM_DT = mybir.dt.float32

N_PAIR = 2           # pipelined s-pairs per core
K_GRP = 8            # k-groups per pair (one input DMA each)


def _build_kernel(n_cores: int):
    """Both matmuls have a 64-wide dim (R) that would leave half the 128x128
    PE array idle. The two 512-row S-tiles of the core's shard are packed
    into the two array halves via tile_position: mm1 puts S-tile 0 in
    columns 0-63 / S-tile 1 in columns 64-127 (outputs land on psum
    partitions 0-63 / 64-127 of the same bank), mm2 puts them in rows
    0-63 / 64-127 (B^T duplicated across both partition halves). Everything
    stays partition-aligned, fp32-exact, and PE time halves twice."""
    nc = bacc.Bacc(
        "TRN2", target_bir_lowering=False, debug=False, num_devices=n_cores
    )
    # xt and at arrive host-packed in SBUF-native layout, so every input
    # DMA reads 8-16KB contiguous per partition (max descriptor efficiency).
    xt = nc.dram_tensor(
        "xt", [N_PAIR, K_GRP, P, KC // K_GRP, S_SHARD // N_PAIR],
        mybir.dt.float32, kind="ExternalInput",
    )
    at = nc.dram_tensor("at", [P, KC, R], mybir.dt.float32, kind="ExternalInput")
    bt = nc.dram_tensor("bt", [R, OUT], mybir.dt.float32, kind="ExternalInput")
    # Output leaves in SBUF-native blocks (dense 512KB writes); the host
    # un-permutes to [S_SHARD, OUT].
    out = nc.dram_tensor(
        "out", [N_PAIR, 2, 2, OC // 2, P, 1024],
        mybir.dt.float32, kind="ExternalOutput",
    )

    KG = K_GRP         # xt k-groups per pair (1 MiB DMAs, 16KB/partition runs)
    KPG = KC // KG     # k-chunks per group
    NPAIR = N_PAIR     # compute pipelined over s-pairs
    SP = S_SHARD // NPAIR   # 512 rows per pair (2 packed tiles of 256)
    STP = SP // 2      # 256: matmul free dim for mm1

    with tile.TileContext(nc) as tc:
        with (
            tc.tile_pool(name="const", bufs=1) as const,
            tc.tile_pool(name="xload", bufs=1) as xload,
            tc.tile_pool(name="xr", bufs=2) as xrp,
            # deep staging so out-DMA keeps draining through mm1 phases
            tc.tile_pool(name="ostage", bufs=6) as ostage,
            tc.tile_pool(name="ps1", bufs=2, space="PSUM") as ps1,
            tc.tile_pool(name="ps2", bufs=3, space="PSUM") as ps2,
        ):
            # All DMAs ride the SP HWDGE ring, which drains FIFO: inputs
            # are issued first (pair 0 before pair 1) and outputs queue
            # behind them, so HBM runs saturated start-to-finish with no
            # in/out bandwidth competition. The small A^T load goes via
            # SWDGE so it doesn't delay the first x group.
            at_sb = const.tile([P, KC, R], MM_DT)
            bt_sb = const.tile([P, OUT], MM_DT)
            nc.gpsimd.dma_start(at_sb[:], at[:])

            xt_t = {}
            for h in range(NPAIR):
                for g in range(KG):
                    xg = xload.tile([P, KPG, SP], MM_DT, tag=f"xt{h}_{g}")
                    nc.sync.dma_start(xg[:], xt[h, g])
                    xt_t[h, g] = xg
                if h == 0:
                    nc.sync.dma_start(bt_sb[0:R, :], bt[:])
                    # duplicate across the second partition half (SBUF fabric,
                    # via SWDGE so it takes no SP-ring slot)
                    nc.gpsimd.dma_start(bt_sb[R : 2 * R, :], bt_sb[0:R, :])

            def mm1_group(h, g, acc):
                """One k-group of mm1 for pair h: 16 column-packed matmuls."""
                for kk in range(KPG):
                    k = g * KPG + kk
                    xk = xt_t[h, g][:, kk, :]
                    nc.tensor.matmul(
                        acc[0:R, :],
                        at_sb[:, k, :],
                        xk[:, 0:STP],
                        start=(k == 0),
                        stop=(k == KC - 1),
                        tile_position=(0, 0),
                    )
                    nc.tensor.matmul(
                        acc[R : 2 * R, :],
                        at_sb[:, k, :],
                        xk[:, STP : 2 * STP],
                        start=(k == 0),
                        stop=(k == KC - 1),
                        tile_position=(0, R),
                    )

            def mm2_block(h, m, n2, xr_sb):
                """One (m, n2) block of mm2 for pair h: 4 row-packed matmuls,
                staged 1024 wide per out-DMA for 4KB DRAM runs."""
                stg0 = ostage.tile([P, 2, 512], mybir.dt.float32, tag="st0")
                stg1 = ostage.tile([P, 2, 512], mybir.dt.float32, tag="st1")
                stg = [stg0, stg1]
                for j in range(2):
                    n = 2 * n2 + j
                    po0 = ps2.tile([P, 512], mybir.dt.float32, tag="po0")
                    po1 = ps2.tile([P, 512], mybir.dt.float32, tag="po1")
                    nc.tensor.matmul(
                        po0[:],
                        xr_sb[0:R, m * P : (m + 1) * P],
                        bt_sb[0:R, n * 512 : (n + 1) * 512],
                        start=True,
                        stop=True,
                        tile_position=(0, 0),
                    )
                    nc.tensor.matmul(
                        po1[:],
                        xr_sb[R : 2 * R, m * P : (m + 1) * P],
                        bt_sb[R : 2 * R, n * 512 : (n + 1) * 512],
                        start=True,
                        stop=True,
                        tile_position=(R, 0),
                    )
                    nc.vector.tensor_copy(stg[0][:, j, :], po0[:])
                    nc.vector.tensor_copy(stg[1][:, j, :], po1[:])
                for t in range(2):
                    nc.sync.dma_start(
                        out[h, m, t, n2],
                        stg[t].rearrange("p a b -> p (a b)"),
                    )

            def xr_copy(acc):
                xr_sb = xrp.tile([P, STP], MM_DT)
                nc.vector.tensor_copy(xr_sb[:], acc[:])
                return xr_sb

            # Pair 0: mm1 paced by its input DMAs.
            acc0 = ps1.tile([P, STP], mybir.dt.float32, tag="acc")
            for g in range(KG):
                mm1_group(0, g, acc0)
            xr0 = xr_copy(acc0)

            # Pair 0's mm2 with pair 1's mm1 k-groups woven in, so after the
            # last input byte lands only mm2(1) remains on the PE.
            acc1 = ps1.tile([P, STP], mybir.dt.float32, tag="acc")
            blocks = [(m, n2) for m in range(STP // P) for n2 in range(OC // 2)]
            for i, (m, n2) in enumerate(blocks):
                mm2_block(0, m, n2, xr0)
                if i < KG:
                    mm1_group(1, i, acc1)
            xr1 = xr_copy(acc1)

            for m, n2 in blocks:
                mm2_block(1, m, n2, xr1)
    nc.compile()
    return nc


_NC_CACHE: dict[int, object] = {}


def _get_nc(n_cores: int):
    if n_cores not in _NC_CACHE:
        _NC_CACHE[n_cores] = _build_kernel(n_cores)
    return _NC_CACHE[n_cores]


def _route_host(x, reliability_vec, regime_vec, w1, b1, w2, b2):
    """Router forward in f32 numpy, matching the jax reference."""
    x32 = np.asarray(x, np.float32)
    x_pooled = x32.mean(axis=1)  # [B, D]
    router_in = np.concatenate(
        [x_pooled, np.asarray(reliability_vec, np.float32),
         np.asarray(regime_vec, np.float32)], axis=-1
    ).astype(np.float32)
    h = router_in @ np.asarray(w1, np.float32) + np.asarray(b1, np.float32)
    # jax.nn.gelu default is the tanh approximation
    h32 = h.astype(np.float32)
    g = 0.5 * h32 * (
        1.0 + np.tanh(
            np.float32(np.sqrt(2.0 / np.pi))
            * (h32 + np.float32(0.044715) * h32 * h32 * h32)
        )
    )
    logits = g.astype(np.float32) @ np.asarray(w2, np.float32) + np.asarray(
        b2, np.float32
    )
    logits = logits.astype(np.float32)
    zmax = logits.max(axis=-1, keepdims=True)
    ez = np.exp(logits - zmax)
    probs = ez / ez.sum(axis=-1, keepdims=True)
    top = np.argmax(probs, axis=-1)
    avg_gate = probs.mean(axis=0)
    balance = np.float32(
        BALANCE_COEFF * (E * np.sum(avg_gate.astype(np.float32) ** 2))
    )
    return top, balance


def _pack_xt(x_shard):
    """x-shard [S_SHARD, D] -> SBUF-native [NPAIR, KG, 128, KPG, SP]:
    element [h, g, p, c, s] = x_shard[h*SP + s, (g*KPG + c)*128 + p]."""
    SP = S_SHARD // N_PAIR
    KPG = KC // K_GRP
    v = x_shard.reshape(N_PAIR, SP, K_GRP, KPG, P)
    return np.ascontiguousarray(v.transpose(0, 2, 4, 3, 1))


def _pack_at(a):
    """A_e [R, D] (scaled) -> [128, KC, R]: [p, c, r] = A^T[c*128+p, r]."""
    v = a.T.reshape(KC, P, R)
    return np.ascontiguousarray(v.transpose(1, 0, 2))


def kernel(x, reliability_vec, regime_vec, lora_A, lora_B, w1, b1, w2, b2):
    x = np.asarray(x, np.float32)
    lora_A = np.asarray(lora_A, np.float32)
    lora_B = np.asarray(lora_B, np.float32)

    top, balance = _route_host(
        x, reliability_vec, regime_vec, w1, b1, w2, b2
    )

    # Per-expert host prep (tiny tensors): A_e^T with LoRA scaling folded in
    # (exact, power of two) and B_e^T.
    at_by_e = {}
    bt_by_e = {}
    for e in set(int(t) for t in top):
        at_by_e[e] = _pack_at(lora_A[e] * np.float32(SCALING))
        bt_by_e[e] = np.ascontiguousarray(lora_B[e].T)

    in_maps = []
    for c in range(N_CORES):
        b, half = divmod(c, N_CORES // B)
        e = int(top[b])
        xt_c = _pack_xt(x[b, half * S_SHARD : (half + 1) * S_SHARD, :])
        in_maps.append({"xt": xt_c, "at": at_by_e[e], "bt": bt_by_e[e]})

    nc = _get_nc(N_CORES)
    res = run_bass_kernel_spmd(nc, in_maps, core_ids=list(range(N_CORES)))

    out = np.empty((B, S, OUT), np.float32)
    for c in range(N_CORES):
        b, half = divmod(c, N_CORES // B)
        # blocks [h, m, t, n2, p, q] -> rows h*512 + t*256 + m*128 + p,
        # cols n2*1024 + q
        v = res.results[c]["out"].transpose(0, 2, 1, 4, 3, 5)
        out[b, half * S_SHARD : (half + 1) * S_SHARD, :] = v.reshape(
            S_SHARD, OUT
        )

    return out, balance
